# revision 26
# baseline (speedup 1.0000x reference)
"""Trainium2 Bass kernel for nn_Attention (batch=8, seq=1024, dim=512, 8 heads x 64).

Strategy: pure data parallelism — one batch element per NeuronCore (8 cores).
No collectives. Per core, everything is computed from a pre-transposed
x^T [512, 1024] so all matmul contractions sit on the partition axis:

  Q^T = wq @ x^T, K^T = wk @ x^T          (d-major, per-head rows)
  S^T[nk, nq] = (K^T)_h.T-slices @ (Q^T)_h  (K=64 contraction)
  E^T = clip(exp(S/8), e^1e-6, e^1)         (exp first: ACT evicts PSUM->SBUF,
                                             then DVE/GpSimd clip, since
                                             exp(clip(x)) == clip'(exp(x)))
  O^T_aug = [V | 1].T @ E^T                 (ones column yields softmax rowsums)
  O^T = O^T_aug[:64] * (1/rowsum)           (recip = ACT exp(-ln(x)); broadcast
                                             via a DRAM-bounce DMA, or a PE
                                             sel-matmul for the last pair)
  y = O^T-slices.T @ wo^T                   (bias added during the GpSimd
                                             eviction from a DMA-broadcast tile)

The kernel is PE-bound: the matmul stream (~392 matmuls, 213ns each at
2.4 GHz for N=512) is the critical path. Schedule goals: start the PE the
moment the sequencer preamble ends (junk matmuls on an early-DMA'd tile
ramp the HAM clock while inputs load), keep the stream gap-free (junk
filler where a dependency wait is unavoidable so the clock never drops),
and keep every non-PE op (exp/clip/evict/normalize) off the PE's path.
"""

import numpy as np
import concourse.bass as bass
import concourse.tile as tile
from concourse import mybir
from concourse.bass_utils import run_bass_kernel_spmd

F32 = mybir.dt.float32
BF16 = mybir.dt.bfloat16

DIM = 512
HEADS = 8
DH = 64
N = 1024
NCORES = 8
SCALE = DH**-0.5
E_LO = float(np.exp(1e-6))
E_HI = float(np.exp(1.0))
EXP = mybir.ActivationFunctionType.Exp
LN = mybir.ActivationFunctionType.Ln
MIN = mybir.AluOpType.min
MAX = mybir.AluOpType.max
ADD = mybir.AluOpType.add
MULT = mybir.AluOpType.mult


def split_multiwait(nc, max_waits=1):
    """Walrus in this env rejects instructions carrying more than one sync
    wait ("Too many sync wait commands" in setupSyncWait). Tile's tail drain
    legitimately accumulates several; split the excess into single-wait NOPs
    inserted just before the offending instruction."""
    nsplit = 0
    for fn in nc.m.functions:
        for bb in fn.blocks:
            insts = list(bb.instructions)
            if not any(
                i.sync_info is not None and len(i.sync_info.on_wait) > max_waits
                for i in insts
            ):
                continue
            new = []
            for i in insts:
                si = i.sync_info
                if si is not None and len(si.on_wait) > max_waits:
                    waits = list(si.on_wait)
                    splittable = [w for w in waits if w.wait_reg is None]
                    keep = [w for w in waits if w.wait_reg is not None]
                    nkeep = max_waits - len(keep)
                    assert nkeep >= 0, "too many register waits to split"
                    tail = splittable[-nkeep:] if nkeep > 0 else []
                    head = splittable[: len(splittable) - len(tail)]
                    for k, w in enumerate(head):
                        nop = mybir.InstNoOp(name=f"{i.name}-sw{k}")
                        nop.engine = i.engine
                        nop.sync_info = mybir.SyncInfo(on_wait=[w], on_update=[])
                        new.append(nop)
                        nsplit += 1
                    i.sync_info = mybir.SyncInfo(
                        on_wait=keep + tail, on_update=list(si.on_update)
                    )
                new.append(i)
            bb.instructions.clear()
            for i in new:
                bb.add_instruction(i)
    return nsplit


def build_nc(et_bufs=48):
    nc = bass.Bass("TRN2")
    xT = nc.dram_tensor("xT", [DIM, N], BF16, kind="ExternalInput")
    wqT = nc.dram_tensor("wqT", [DIM, DIM], BF16, kind="ExternalInput")
    wkT = nc.dram_tensor("wkT", [DIM, DIM], BF16, kind="ExternalInput")
    wvT = nc.dram_tensor("wvT", [DIM, DIM], BF16, kind="ExternalInput")
    woT = nc.dram_tensor("woT", [DIM, DIM], BF16, kind="ExternalInput")
    bob = nc.dram_tensor("bob", [1, DIM], BF16, kind="ExternalInput")
    out = nc.dram_tensor("out", [N, DIM], F32, kind="ExternalOutput")

    with tile.TileContext(nc) as tc:
        with (
            tc.tile_pool(name="consts", bufs=1) as consts,
            tc.tile_pool(name="etp", bufs=et_bufs) as etp,
            tc.tile_pool(name="rp", bufs=2) as rp,
            tc.tile_pool(name="yp", bufs=5) as yp,
            tc.tile_pool(name="drp", bufs=8, space="DRAM") as drp,
            # tag "m" is shared by the [128,512] proj tiles and [65,512] attn
            # tiles (same 2KB/partition) so QKV/attn/final all fit in 8 banks
            tc.tile_pool(name="pp_mix", bufs=4, space="PSUM") as pp_mix,
            tc.tile_pool(name="pp_st", bufs=2, space="PSUM") as pp_st,
        ):
            # ---- input DMAs, critical-first ------------------------------
            # xt/wq/wk gate the first projections; wv/wo/bob only matter
            # later. jt (the junk-matmul feed) comes from a GpSimd memset,
            # NOT a DMA: a DMA completing before the engines' semaphore-init
            # preamble (~6us) has its completion update wiped, so a matmul
            # waiting on it stalls until the next DMA on the pooled
            # semaphore. Pool's main starts right when junk should.
            jt = consts.tile([128, 128], BF16, name="jt", tag="jt")
            nc.gpsimd.memset(jt, 0.25)

            def load4(src, cols, nm, eng):
                tiles = []
                for k in range(4):
                    t = consts.tile([128, cols], BF16, name=f"{nm}{k}", tag=f"{nm}{k}")
                    eng.dma_start(t, src[k * 128 : (k + 1) * 128, :])
                    tiles.append(t)
                return tiles

            # Input transfers only start flowing ~9us in (static-DMA trigger)
            # and the 8 cores share HBM bandwidth, so the whole input phase
            # streams for ~12us. What matters is BYTE ORDER: st_pair(0) needs
            # xt plus only the pair-0 column slices of wq/wk (~1.26MB), so
            # wq/wk arrive as per-pair slices, pair order, on their queues.
            xt = load4(xT, N, "xt", nc.sync)
            wq_t = [[None] * 4 for _ in range(4)]
            wk_t = [[None] * 4 for _ in range(4)]
            for p in range(4):
                for k in range(4):
                    for nm, src, dst, eng in (
                        ("wq", wqT, wq_t, nc.scalar),
                        ("wk", wkT, wk_t, nc.gpsimd),
                    ):
                        t = consts.tile(
                            [128, 128], BF16, name=f"{nm}{p}_{k}", tag=f"{nm}{p}_{k}"
                        )
                        eng.dma_start(
                            t, src[k * 128 : (k + 1) * 128, p * 128 : (p + 1) * 128]
                        )
                        dst[p][k] = t

            # ---- PE warm-up + ACT table preload --------------------------
            # Junk matmuls on jt start the instant the PE sequencer finishes
            # its preamble (jt's DMA lands well before that) so the HAM
            # clock-gate ramps to 2.4 GHz while the input DMAs drain. A tiny
            # exp on jt triggers the one-time ACT_TABLE_LOAD up front.
            def junk(n, tag="ps_st"):
                pool = pp_st if tag == "ps_st" else pp_mix
                for _ in range(n):
                    ps = pool.tile([128, 128], F32, name="junk", tag=tag)
                    nc.tensor.matmul(ps, lhsT=jt, rhs=jt, start=True, stop=True)

            junk(32)
            wu_e = consts.tile([128, 1], BF16, name="wu_e", tag="wu_e")
            nc.scalar.activation(wu_e, jt[:, 0:1], EXP, scale=SCALE)

            wv = load4(wvT, DIM, "wv", nc.gpsimd)
            wo = load4(woT, DIM, "wo", nc.gpsimd)
            # bias broadcast to all partitions (DRAM partition-broadcast read)
            bob_ap = bob[:, :]
            bo_bc = consts.tile([128, DIM], BF16, name="bo_bc", tag="bo_bc")
            nc.gpsimd.dma_start(
                bo_bc,
                bass.AP(
                    tensor=bob_ap.tensor,
                    offset=bob_ap.offset,
                    ap=[[0, 128]] + list(bob_ap.ap)[1:],
                ),
            )
            # sel rows for the pair-3 PE partition-broadcast (rows 0 and 32)
            sel = consts.tile([128, 128], F32, name="sel", tag="sel")
            nc.vector.memset(sel, 0.0)
            nc.vector.memset(sel[0:1, 0:64], 1.0)
            nc.vector.memset(sel[32:33, 64:128], 1.0)

            # ---- phase bodies as closures, emitted in interleaved order ----
            qT, kT = [None] * 4, [None] * 4
            vaug = [None] * 8
            et = {}
            oT = []
            for p in range(4):
                o = consts.tile([128, N], BF16, name=f"oT{p}", tag=f"oT{p}")
                oT.append(o)

            def proj_qk(p):
                q = consts.tile([128, N], BF16, name=f"qT{p}", tag=f"qT{p}")
                k_ = consts.tile([128, N], BF16, name=f"kT{p}", tag=f"kT{p}")
                qT[p], kT[p] = q, k_
                for c in range(2):
                    for wsb, dst in ((wq_t[p], q), (wk_t[p], k_)):
                        ps = pp_mix.tile([128, 512], F32, name="ps_proj", tag="m")
                        for k in range(4):
                            nc.tensor.matmul(
                                ps,
                                lhsT=wsb[k],
                                rhs=xt[k][:, c * 512 : (c + 1) * 512],
                                start=(k == 0),
                                stop=(k == 3),
                            )
                        nc.vector.tensor_copy(dst[:, c * 512 : (c + 1) * 512], ps)

            def proj_v(j):
                va = consts.tile([128, HEADS * 65], BF16, name=f"va{j}", tag=f"va{j}")
                vaug[j] = va
                ps = pp_mix.tile([128, 512], F32, name="ps_proj", tag="m")
                for k in range(4):
                    nc.tensor.matmul(
                        ps,
                        lhsT=xt[k][:, j * 128 : (j + 1) * 128],
                        rhs=wv[k],
                        start=(k == 0),
                        stop=(k == 3),
                    )
                nc.gpsimd.memset(va, 1.0)
                va3 = va.rearrange("p (h c) -> p h c", c=65)
                nc.vector.tensor_copy(
                    va3[:, :, 0:64], ps.rearrange("p (h c) -> p h c", c=64)
                )

            def st_pair(p):
                # S^T -> exp (ACT, also evicts PSUM) -> clip (DVE/GpSimd)
                for j in range(8):
                    for hh in range(2):
                        h = 2 * p + hh
                        ps = pp_st.tile([128, N], F32, name="ps_st", tag="ps_st")
                        for c in range(2):
                            nc.tensor.matmul(
                                ps[:, c * 512 : (c + 1) * 512],
                                lhsT=kT[p][hh * 64 : (hh + 1) * 64, j * 128 : (j + 1) * 128],
                                rhs=qT[p][hh * 64 : (hh + 1) * 64, c * 512 : (c + 1) * 512],
                                start=True,
                                stop=True,
                            )
                        e = etp.tile([128, N], BF16, name="et", tag="et")
                        nc.scalar.activation(e, ps, EXP, scale=SCALE)
                        ceng = nc.gpsimd if (j + hh) % 3 == 2 else nc.vector
                        ceng.tensor_scalar(e, e, E_HI, E_LO, MIN, MAX)
                        et[(h, j)] = e

            attn_state = {}

            def attn_chains(p):
                # matmuls + rowsum gather + unnormalized-O eviction only.
                # The recip/normalize half is emitted AFTER the next pair's
                # S^T exps: ACT is strict FIFO, so a reciprocal waiting on
                # rowsums must not sit in front of ready exps.
                groups = [(0, 1)] if p < 3 else [(0,), (1,)]
                saved = []
                for grp in groups:
                    rs = rp.tile([128, 512], F32, name="rs", tag="rs")
                    nc.gpsimd.memset(rs, 1.0)
                    chains = []
                    for hh in grp:
                        h = 2 * p + hh
                        for c in range(2):
                            l = (hh - grp[0]) * 2 + c
                            oa = pp_mix.tile([65, 512], F32, name="ps_oa", tag="m")
                            for j in range(8):
                                nc.tensor.matmul(
                                    oa,
                                    lhsT=vaug[j][:, h * 65 : (h + 1) * 65],
                                    rhs=et[(h, j)][:, c * 512 : (c + 1) * 512],
                                    start=(j == 0),
                                    stop=(j == 7),
                                )
                            nc.vector.tensor_copy(
                                rs[32 * l : 32 * l + 1, :], oa[64:65, :]
                            )
                            if p == 3:
                                # keep oa resident in PSUM; normalized in
                                # attn_finish straight from the bank
                                chains.append((oa[0:64, :], hh, c, l))
                            else:
                                o_un = rp.tile(
                                    [64, 512], F32, name="o_un", tag="o_un", bufs=6
                                )
                                nc.vector.tensor_copy(o_un, oa[0:64, :])
                                chains.append((o_un, hh, c, l))
                    saved.append((rs, grp, chains))
                attn_state[p] = saved

            def attn_finish(p):
                for rs, grp, chains in attn_state[p]:
                    nrows = 32 * len(grp) * 2
                    nc.scalar.activation(rs[0:nrows, :], rs[0:nrows, :], LN)
                    rinv = rp.tile([128, 512], F32, name="rinv", tag="rinv")
                    nc.scalar.activation(
                        rinv[0:nrows, :], rs[0:nrows, :], EXP, scale=-1.0
                    )
                    for o_un, hh, c, l in chains:
                        dst = oT[p][hh * 64 : (hh + 1) * 64, c * 512 : (c + 1) * 512]
                        if p == 3:
                            # PE broadcast of rinv row -> [64,512]. bc takes
                            # the mix slot the o_sb copy just freed (the
                            # ps_st slots are held by the final_pre tiles).
                            # An engine op can read only one PSUM operand, so
                            # hop the unnormalized O through SBUF first.
                            bc = pp_mix.tile([64, 512], F32, name="bc_ps", tag="m")
                            nc.tensor.matmul(
                                bc,
                                lhsT=sel[:, l * 64 : (l + 1) * 64],
                                rhs=rinv,
                                start=True,
                                stop=True,
                            )
                            o_sb = rp.tile([64, 512], F32, name="o_sb", tag="o_un", bufs=6)
                            nc.vector.tensor_copy(o_sb, o_un)
                            nc.vector.tensor_mul(dst, o_sb, bc)
                        else:
                            rd = drp.tile([1, 512], F32, name="rd", tag="rd")
                            nc.sync.dma_start(rd, rinv[32 * l : 32 * l + 1, :])
                            rb = rp.tile([64, 512], F32, name="rb", tag="rb", bufs=4)
                            nc.sync.dma_start(
                                rb,
                                bass.AP(
                                    tensor=rd.tensor,
                                    offset=rd.offset,
                                    ap=[[0, 64]] + list(rd[:].ap)[1:],
                                ),
                            )
                            nc.gpsimd.tensor_mul(dst, o_un, rb)

            out_qs = [nc.sync, nc.gpsimd, nc.scalar]
            fin_pre = {}

            def final_pre(t):
                # k=0..2 accumulated early (oT[0..2] are ready well before
                # pair 3 finishes) into the ps_st banks, which the last
                # exps have drained by now; only k=3 + evict stay on the tail
                ps = pp_st.tile([128, 512], F32, name="ps_fin", tag="ps_st")
                for k in range(3):
                    nc.tensor.matmul(
                        ps,
                        lhsT=oT[k][:, t * 128 : (t + 1) * 128],
                        rhs=wo[k],
                        start=(k == 0),
                        stop=False,
                    )
                fin_pre[t] = ps

            def final_fin(t):
                ps = fin_pre[t]
                nc.tensor.matmul(
                    ps,
                    lhsT=oT[3][:, t * 128 : (t + 1) * 128],
                    rhs=wo[3],
                    start=False,
                    stop=True,
                )
                y = yp.tile([128, 512], F32, name="y", tag="y")
                nc.vector.tensor_add(y, ps, bo_bc)
                out_qs[t % 3].dma_start(out[t * 128 : (t + 1) * 128, :], y)

            def final_proj(t):
                # alternate PSUM pools (both free in the tail) so four final
                # chains run concurrently instead of two; k<3 accumulations
                # and the bias matmul come before the k=3 step, which is the
                # one gated on the last pair's normalize. The output DMA
                # reads the finished tile straight out of PSUM — no engine
                # eviction on the tail at all.
                if t % 2 == 0:
                    ps = pp_st.tile([128, 512], F32, name="ps_fin", tag="ps_st")
                else:
                    ps = pp_mix.tile([128, 512], F32, name="ps_fin", tag="m")
                for k in range(4):
                    nc.tensor.matmul(
                        ps,
                        lhsT=oT[k][:, t * 128 : (t + 1) * 128],
                        rhs=wo[k],
                        start=(k == 0),
                        stop=(k == 3),
                    )
                y = yp.tile([128, 512], F32, name="y", tag="y")
                # bias add fused into the eviction (DVE: GpSimd can't read
                # PSUM, and ACT must stay clear for the exp/recip stream)
                nc.vector.tensor_add(y, ps, bo_bc)
                out_qs[t % 3].dma_start(out[t * 128 : (t + 1) * 128, :], y)

            # ---- interleaved emission --------------------------------------
            # S^T(0) is emitted immediately after its own Q/K projection so
            # the ACT exp cadence starts early; the remaining projections
            # fill PE slack under the ACT-bound phase.
            proj_qk(0)
            st_pair(0)
            proj_qk(1)
            for j in range(8):
                proj_v(j)
            st_pair(1)
            attn_chains(0)
            attn_finish(0)
            proj_qk(2)
            st_pair(2)
            proj_qk(3)
            attn_chains(1)
            attn_finish(1)
            st_pair(3)
            attn_chains(2)
            attn_finish(2)
            attn_chains(3)
            # real work as clock-keeping filler while pair-3's reciprocal
            # chain (rowsums -> ln -> exp -> sel broadcast) resolves
            final_pre(0)
            final_pre(1)
            attn_finish(3)
            final_fin(0)
            final_fin(1)
            for t in range(2, 8):
                final_proj(t)

    split_multiwait(nc)
    return nc


_NC = None


def _get_nc():
    global _NC
    if _NC is None:
        _NC = build_nc()
    return _NC


def make_in_maps(x, wq, wk, wv, wo, bo):
    bf = mybir.dt.np(BF16)
    shared = {
        "wqT": np.ascontiguousarray(wq.T).astype(bf),
        "wkT": np.ascontiguousarray(wk.T).astype(bf),
        "wvT": np.ascontiguousarray(wv.T).astype(bf),
        "woT": np.ascontiguousarray(wo.T).astype(bf),
        "bob": np.asarray(bo, dtype=np.float32).reshape(1, -1).astype(bf),
    }
    xT_all = np.ascontiguousarray(x.transpose(0, 2, 1)).astype(bf)
    return [{"xT": xT_all[b], **shared} for b in range(NCORES)]


def run(x, wq, wk, wv, wo, bo, **spmd_kwargs):
    nc = _get_nc()
    in_maps = make_in_maps(
        np.asarray(x), np.asarray(wq), np.asarray(wk),
        np.asarray(wv), np.asarray(wo), np.asarray(bo),
    )
    res = run_bass_kernel_spmd(nc, in_maps, core_ids=list(range(NCORES)), **spmd_kwargs)
    out = np.stack([res.results[b]["out"] for b in range(NCORES)], axis=0)
    return out.astype(np.float32), res


def kernel(x, wq, wk, wv, wo, bo):
    out, _ = run(x, wq, wk, wv, wo, bo)
    return out


# revision 28
# speedup vs baseline: 1.0099x; 1.0099x over previous
"""Trainium2 Bass kernel for nn_Attention (batch=8, seq=1024, dim=512, 8 heads x 64).

Strategy: pure data parallelism — one batch element per NeuronCore (8 cores).
No collectives. Per core, everything is computed from a pre-transposed
x^T [512, 1024] so all matmul contractions sit on the partition axis:

  Q^T = wq @ x^T, K^T = wk @ x^T          (d-major, per-head rows)
  S^T[nk, nq] = (K^T)_h.T-slices @ (Q^T)_h  (K=64 contraction)
  E^T = clip(exp(S/8), e^1e-6, e^1)         (exp first: ACT evicts PSUM->SBUF,
                                             then DVE/GpSimd clip, since
                                             exp(clip(x)) == clip'(exp(x)))
  O^T_aug = [V | 1].T @ E^T                 (ones column yields softmax rowsums)
  O^T = O^T_aug[:64] * (1/rowsum)           (recip = ACT exp(-ln(x)); broadcast
                                             via a DRAM-bounce DMA, or a PE
                                             sel-matmul for the last pair)
  y = O^T-slices.T @ wo^T                   (bias added during the GpSimd
                                             eviction from a DMA-broadcast tile)

The kernel is PE-bound: the matmul stream (~392 matmuls, ~216ns each at
2.4 GHz for N=512, LD_WEIGHTS hidden) is the critical path. Schedule:
- PE warm-up junk feeds off a GpSimd-memset tile, NOT a DMA: a DMA
  completing before the engines' semaphore-init preamble (~6us) has its
  completion update wiped, so its waiter stalls until the next DMA on the
  pooled semaphore (+2.5us). Junk runs from ~7.8us and holds the HAM
  clock at speed while inputs stream in.
- Input DMAs only flow from ~9us (static-DMA trigger) and all 8 cores
  share HBM, so the input phase is ~12us of streaming; BYTE ORDER is what
  matters. wq/wk arrive as per-pair column slices in pair order, so
  st_pair(0) is gated on ~1.26MB instead of 2.5MB.
- Tail: final-projection tiles 0-1 pre-accumulate k=0..2 into the ps_st
  banks (drained by the last exps) exactly in the window where the PE
  would otherwise wait for pair-3's reciprocal; only the k=3 matmul and
  the eviction stay gated on oT[3]. Bias is added during the DVE
  eviction from a DMA-broadcast bias tile (no K=1 bias matmuls), and
  output DMAs rotate across three queues.
Measured on trn2: 128.6-128.8us typical HW exec (vs 130.4us baseline;
the machine drifts between a ~128us and a ~152us process-level mode —
comparisons are only valid within one process), rel err 4.3e-3.
"""

import numpy as np
import concourse.bass as bass
import concourse.tile as tile
from concourse import mybir
from concourse.bass_utils import run_bass_kernel_spmd

F32 = mybir.dt.float32
BF16 = mybir.dt.bfloat16

DIM = 512
HEADS = 8
DH = 64
N = 1024
NCORES = 8
SCALE = DH**-0.5
E_LO = float(np.exp(1e-6))
E_HI = float(np.exp(1.0))
EXP = mybir.ActivationFunctionType.Exp
LN = mybir.ActivationFunctionType.Ln
MIN = mybir.AluOpType.min
MAX = mybir.AluOpType.max
ADD = mybir.AluOpType.add
MULT = mybir.AluOpType.mult


def split_multiwait(nc, max_waits=1):
    """Walrus in this env rejects instructions carrying more than one sync
    wait ("Too many sync wait commands" in setupSyncWait). Tile's tail drain
    legitimately accumulates several; split the excess into single-wait NOPs
    inserted just before the offending instruction."""
    nsplit = 0
    for fn in nc.m.functions:
        for bb in fn.blocks:
            insts = list(bb.instructions)
            if not any(
                i.sync_info is not None and len(i.sync_info.on_wait) > max_waits
                for i in insts
            ):
                continue
            new = []
            for i in insts:
                si = i.sync_info
                if si is not None and len(si.on_wait) > max_waits:
                    waits = list(si.on_wait)
                    splittable = [w for w in waits if w.wait_reg is None]
                    keep = [w for w in waits if w.wait_reg is not None]
                    nkeep = max_waits - len(keep)
                    assert nkeep >= 0, "too many register waits to split"
                    tail = splittable[-nkeep:] if nkeep > 0 else []
                    head = splittable[: len(splittable) - len(tail)]
                    for k, w in enumerate(head):
                        nop = mybir.InstNoOp(name=f"{i.name}-sw{k}")
                        nop.engine = i.engine
                        nop.sync_info = mybir.SyncInfo(on_wait=[w], on_update=[])
                        new.append(nop)
                        nsplit += 1
                    i.sync_info = mybir.SyncInfo(
                        on_wait=keep + tail, on_update=list(si.on_update)
                    )
                new.append(i)
            bb.instructions.clear()
            for i in new:
                bb.add_instruction(i)
    return nsplit


def build_nc(et_bufs=48):
    nc = bass.Bass("TRN2")
    xT = nc.dram_tensor("xT", [DIM, N], BF16, kind="ExternalInput")
    wqT = nc.dram_tensor("wqT", [DIM, DIM], BF16, kind="ExternalInput")
    wkT = nc.dram_tensor("wkT", [DIM, DIM], BF16, kind="ExternalInput")
    wvT = nc.dram_tensor("wvT", [DIM, DIM], BF16, kind="ExternalInput")
    woT = nc.dram_tensor("woT", [DIM, DIM], BF16, kind="ExternalInput")
    bob = nc.dram_tensor("bob", [1, DIM], BF16, kind="ExternalInput")
    out = nc.dram_tensor("out", [N, DIM], F32, kind="ExternalOutput")

    with tile.TileContext(nc) as tc:
        with (
            tc.tile_pool(name="consts", bufs=1) as consts,
            tc.tile_pool(name="etp", bufs=et_bufs) as etp,
            tc.tile_pool(name="rp", bufs=2) as rp,
            tc.tile_pool(name="yp", bufs=5) as yp,
            tc.tile_pool(name="drp", bufs=8, space="DRAM") as drp,
            # tag "m" is shared by the [128,512] proj tiles and [65,512] attn
            # tiles (same 2KB/partition) so QKV/attn/final all fit in 8 banks
            tc.tile_pool(name="pp_mix", bufs=4, space="PSUM") as pp_mix,
            tc.tile_pool(name="pp_st", bufs=2, space="PSUM") as pp_st,
        ):
            # ---- input DMAs, critical-first ------------------------------
            # xt/wq/wk gate the first projections; wv/wo/bob only matter
            # later. jt (the junk-matmul feed) comes from a GpSimd memset,
            # NOT a DMA: a DMA completing before the engines' semaphore-init
            # preamble (~6us) has its completion update wiped, so a matmul
            # waiting on it stalls until the next DMA on the pooled
            # semaphore. Pool's main starts right when junk should.
            jt = consts.tile([128, 128], BF16, name="jt", tag="jt")
            nc.gpsimd.memset(jt, 0.25)

            def load4(src, cols, nm, eng):
                tiles = []
                for k in range(4):
                    t = consts.tile([128, cols], BF16, name=f"{nm}{k}", tag=f"{nm}{k}")
                    eng.dma_start(t, src[k * 128 : (k + 1) * 128, :])
                    tiles.append(t)
                return tiles

            # Input transfers only start flowing ~9us in (static-DMA trigger)
            # and the 8 cores share HBM bandwidth, so the whole input phase
            # streams for ~12us. What matters is BYTE ORDER: st_pair(0) needs
            # xt plus only the pair-0 column slices of wq/wk (~1.26MB), so
            # wq/wk arrive as per-pair slices, pair order, on their queues.
            # wq rides the SP queue with xt: the Activation sequencer must
            # stay EMPTY before wu_e, or its serial ~667ns-per-dma issue
            # time pushes the exp-table load (and with it the whole exp
            # stream) out by ~10us.
            xt = load4(xT, N, "xt", nc.sync)
            wq_t = [[None] * 4 for _ in range(4)]
            wk_t = [[None] * 4 for _ in range(4)]
            for p in range(4):
                for k in range(4):
                    for nm, src, dst, eng in (
                        ("wq", wqT, wq_t, nc.sync),
                        ("wk", wkT, wk_t, nc.gpsimd),
                    ):
                        t = consts.tile(
                            [128, 128], BF16, name=f"{nm}{p}_{k}", tag=f"{nm}{p}_{k}"
                        )
                        eng.dma_start(
                            t, src[k * 128 : (k + 1) * 128, p * 128 : (p + 1) * 128]
                        )
                        dst[p][k] = t

            # ---- PE warm-up + ACT table preload --------------------------
            # Junk matmuls on jt start the instant the PE sequencer finishes
            # its preamble (jt's DMA lands well before that) so the HAM
            # clock-gate ramps to 2.4 GHz while the input DMAs drain. A tiny
            # exp on jt triggers the one-time ACT_TABLE_LOAD up front.
            def junk(n, tag="ps_st"):
                pool = pp_st if tag == "ps_st" else pp_mix
                for _ in range(n):
                    ps = pool.tile([128, 128], F32, name="junk", tag=tag)
                    nc.tensor.matmul(ps, lhsT=jt, rhs=jt, start=True, stop=True)

            junk(32)
            wu_e = consts.tile([128, 1], BF16, name="wu_e", tag="wu_e")
            nc.scalar.activation(wu_e, jt[:, 0:1], EXP, scale=SCALE)

            wv = load4(wvT, DIM, "wv", nc.gpsimd)
            wo = load4(woT, DIM, "wo", nc.gpsimd)
            # bias broadcast to all partitions (DRAM partition-broadcast read)
            bob_ap = bob[:, :]
            bo_bc = consts.tile([128, DIM], BF16, name="bo_bc", tag="bo_bc")
            nc.gpsimd.dma_start(
                bo_bc,
                bass.AP(
                    tensor=bob_ap.tensor,
                    offset=bob_ap.offset,
                    ap=[[0, 128]] + list(bob_ap.ap)[1:],
                ),
            )
            # sel rows for the pair-3 PE partition-broadcast (rows 0 and 32)
            sel = consts.tile([128, 128], F32, name="sel", tag="sel")
            nc.vector.memset(sel, 0.0)
            nc.vector.memset(sel[0:1, 0:64], 1.0)
            nc.vector.memset(sel[32:33, 64:128], 1.0)

            # ---- phase bodies as closures, emitted in interleaved order ----
            qT, kT = [None] * 4, [None] * 4
            vaug = [None] * 8
            et = {}
            oT = []
            for p in range(4):
                o = consts.tile([128, N], BF16, name=f"oT{p}", tag=f"oT{p}")
                oT.append(o)

            def proj_qk(p):
                q = consts.tile([128, N], BF16, name=f"qT{p}", tag=f"qT{p}")
                k_ = consts.tile([128, N], BF16, name=f"kT{p}", tag=f"kT{p}")
                qT[p], kT[p] = q, k_
                for c in range(2):
                    for wsb, dst in ((wq_t[p], q), (wk_t[p], k_)):
                        ps = pp_mix.tile([128, 512], F32, name="ps_proj", tag="m")
                        for k in range(4):
                            nc.tensor.matmul(
                                ps,
                                lhsT=wsb[k],
                                rhs=xt[k][:, c * 512 : (c + 1) * 512],
                                start=(k == 0),
                                stop=(k == 3),
                            )
                        nc.vector.tensor_copy(dst[:, c * 512 : (c + 1) * 512], ps)

            def proj_v(j):
                va = consts.tile([128, HEADS * 65], BF16, name=f"va{j}", tag=f"va{j}")
                vaug[j] = va
                ps = pp_mix.tile([128, 512], F32, name="ps_proj", tag="m")
                for k in range(4):
                    nc.tensor.matmul(
                        ps,
                        lhsT=xt[k][:, j * 128 : (j + 1) * 128],
                        rhs=wv[k],
                        start=(k == 0),
                        stop=(k == 3),
                    )
                nc.gpsimd.memset(va, 1.0)
                va3 = va.rearrange("p (h c) -> p h c", c=65)
                nc.vector.tensor_copy(
                    va3[:, :, 0:64], ps.rearrange("p (h c) -> p h c", c=64)
                )

            def st_pair(p):
                # S^T -> exp (ACT, also evicts PSUM) -> clip (DVE/GpSimd)
                for j in range(8):
                    for hh in range(2):
                        h = 2 * p + hh
                        ps = pp_st.tile([128, N], F32, name="ps_st", tag="ps_st")
                        for c in range(2):
                            nc.tensor.matmul(
                                ps[:, c * 512 : (c + 1) * 512],
                                lhsT=kT[p][hh * 64 : (hh + 1) * 64, j * 128 : (j + 1) * 128],
                                rhs=qT[p][hh * 64 : (hh + 1) * 64, c * 512 : (c + 1) * 512],
                                start=True,
                                stop=True,
                            )
                        e = etp.tile([128, N], BF16, name="et", tag="et")
                        nc.scalar.activation(e, ps, EXP, scale=SCALE)
                        ceng = nc.gpsimd if (j + hh) % 3 == 2 else nc.vector
                        ceng.tensor_scalar(e, e, E_HI, E_LO, MIN, MAX)
                        et[(h, j)] = e

            attn_state = {}

            def attn_chains(p):
                # matmuls + rowsum gather + unnormalized-O eviction only.
                # The recip/normalize half is emitted AFTER the next pair's
                # S^T exps: ACT is strict FIFO, so a reciprocal waiting on
                # rowsums must not sit in front of ready exps.
                groups = [(0, 1)] if p < 3 else [(0,), (1,)]
                saved = []
                for grp in groups:
                    rs = rp.tile([128, 512], F32, name="rs", tag="rs")
                    nc.gpsimd.memset(rs, 1.0)
                    chains = []
                    for hh in grp:
                        h = 2 * p + hh
                        for c in range(2):
                            l = (hh - grp[0]) * 2 + c
                            oa = pp_mix.tile([65, 512], F32, name="ps_oa", tag="m")
                            for j in range(8):
                                nc.tensor.matmul(
                                    oa,
                                    lhsT=vaug[j][:, h * 65 : (h + 1) * 65],
                                    rhs=et[(h, j)][:, c * 512 : (c + 1) * 512],
                                    start=(j == 0),
                                    stop=(j == 7),
                                )
                            nc.vector.tensor_copy(
                                rs[32 * l : 32 * l + 1, :], oa[64:65, :]
                            )
                            if p == 3:
                                # keep oa resident in PSUM; normalized in
                                # attn_finish straight from the bank
                                chains.append((oa[0:64, :], hh, c, l))
                            else:
                                o_un = rp.tile(
                                    [64, 512], F32, name="o_un", tag="o_un", bufs=6
                                )
                                nc.vector.tensor_copy(o_un, oa[0:64, :])
                                chains.append((o_un, hh, c, l))
                    saved.append((rs, grp, chains))
                attn_state[p] = saved

            def attn_finish(p):
                for rs, grp, chains in attn_state[p]:
                    nrows = 32 * len(grp) * 2
                    nc.scalar.activation(rs[0:nrows, :], rs[0:nrows, :], LN)
                    rinv = rp.tile([128, 512], F32, name="rinv", tag="rinv")
                    nc.scalar.activation(
                        rinv[0:nrows, :], rs[0:nrows, :], EXP, scale=-1.0
                    )
                    for o_un, hh, c, l in chains:
                        dst = oT[p][hh * 64 : (hh + 1) * 64, c * 512 : (c + 1) * 512]
                        if p == 3:
                            # PE broadcast of rinv row -> [64,512]. bc takes
                            # the mix slot the o_sb copy just freed (the
                            # ps_st slots are held by the final_pre tiles).
                            # An engine op can read only one PSUM operand, so
                            # hop the unnormalized O through SBUF first.
                            bc = pp_mix.tile([64, 512], F32, name="bc_ps", tag="m")
                            nc.tensor.matmul(
                                bc,
                                lhsT=sel[:, l * 64 : (l + 1) * 64],
                                rhs=rinv,
                                start=True,
                                stop=True,
                            )
                            o_sb = rp.tile([64, 512], F32, name="o_sb", tag="o_un", bufs=6)
                            nc.vector.tensor_copy(o_sb, o_un)
                            nc.vector.tensor_mul(dst, o_sb, bc)
                        else:
                            rd = drp.tile([1, 512], F32, name="rd", tag="rd")
                            nc.sync.dma_start(rd, rinv[32 * l : 32 * l + 1, :])
                            rb = rp.tile([64, 512], F32, name="rb", tag="rb", bufs=4)
                            nc.sync.dma_start(
                                rb,
                                bass.AP(
                                    tensor=rd.tensor,
                                    offset=rd.offset,
                                    ap=[[0, 64]] + list(rd[:].ap)[1:],
                                ),
                            )
                            nc.gpsimd.tensor_mul(dst, o_un, rb)

            out_qs = [nc.sync, nc.gpsimd, nc.scalar]
            fin_pre = {}

            def final_pre(t):
                # k=0..2 accumulated early (oT[0..2] are ready well before
                # pair 3 finishes) into the ps_st banks, which the last
                # exps have drained by now; only k=3 + evict stay on the tail
                ps = pp_st.tile([128, 512], F32, name="ps_fin", tag="ps_st")
                for k in range(3):
                    nc.tensor.matmul(
                        ps,
                        lhsT=oT[k][:, t * 128 : (t + 1) * 128],
                        rhs=wo[k],
                        start=(k == 0),
                        stop=False,
                    )
                fin_pre[t] = ps

            def final_fin(t):
                ps = fin_pre[t]
                nc.tensor.matmul(
                    ps,
                    lhsT=oT[3][:, t * 128 : (t + 1) * 128],
                    rhs=wo[3],
                    start=False,
                    stop=True,
                )
                y = yp.tile([128, 512], F32, name="y", tag="y")
                nc.vector.tensor_add(y, ps, bo_bc)
                out_qs[t % 3].dma_start(out[t * 128 : (t + 1) * 128, :], y)

            def final_proj(t):
                # alternate PSUM pools (both free in the tail) so four final
                # chains run concurrently instead of two; k<3 accumulations
                # and the bias matmul come before the k=3 step, which is the
                # one gated on the last pair's normalize. The output DMA
                # reads the finished tile straight out of PSUM — no engine
                # eviction on the tail at all.
                if t % 2 == 0:
                    ps = pp_st.tile([128, 512], F32, name="ps_fin", tag="ps_st")
                else:
                    ps = pp_mix.tile([128, 512], F32, name="ps_fin", tag="m")
                for k in range(4):
                    nc.tensor.matmul(
                        ps,
                        lhsT=oT[k][:, t * 128 : (t + 1) * 128],
                        rhs=wo[k],
                        start=(k == 0),
                        stop=(k == 3),
                    )
                y = yp.tile([128, 512], F32, name="y", tag="y")
                # bias add fused into the eviction (DVE: GpSimd can't read
                # PSUM, and ACT must stay clear for the exp/recip stream)
                nc.vector.tensor_add(y, ps, bo_bc)
                out_qs[t % 3].dma_start(out[t * 128 : (t + 1) * 128, :], y)

            # ---- interleaved emission --------------------------------------
            # S^T(0) is emitted immediately after its own Q/K projection so
            # the ACT exp cadence starts early; the remaining projections
            # fill PE slack under the ACT-bound phase.
            proj_qk(0)
            st_pair(0)
            proj_qk(1)
            for j in range(8):
                proj_v(j)
            st_pair(1)
            attn_chains(0)
            attn_finish(0)
            proj_qk(2)
            st_pair(2)
            proj_qk(3)
            attn_chains(1)
            attn_finish(1)
            st_pair(3)
            attn_chains(2)
            attn_finish(2)
            attn_chains(3)
            # real work as clock-keeping filler while pair-3's reciprocal
            # chain (rowsums -> ln -> exp -> sel broadcast) resolves
            final_pre(0)
            final_pre(1)
            attn_finish(3)
            final_fin(0)
            final_fin(1)
            for t in range(2, 8):
                final_proj(t)

    split_multiwait(nc)
    return nc


_NC = None


def _get_nc():
    global _NC
    if _NC is None:
        _NC = build_nc()
    return _NC


def make_in_maps(x, wq, wk, wv, wo, bo):
    bf = mybir.dt.np(BF16)
    shared = {
        "wqT": np.ascontiguousarray(wq.T).astype(bf),
        "wkT": np.ascontiguousarray(wk.T).astype(bf),
        "wvT": np.ascontiguousarray(wv.T).astype(bf),
        "woT": np.ascontiguousarray(wo.T).astype(bf),
        "bob": np.asarray(bo, dtype=np.float32).reshape(1, -1).astype(bf),
    }
    xT_all = np.ascontiguousarray(x.transpose(0, 2, 1)).astype(bf)
    return [{"xT": xT_all[b], **shared} for b in range(NCORES)]


def run(x, wq, wk, wv, wo, bo, **spmd_kwargs):
    nc = _get_nc()
    in_maps = make_in_maps(
        np.asarray(x), np.asarray(wq), np.asarray(wk),
        np.asarray(wv), np.asarray(wo), np.asarray(bo),
    )
    res = run_bass_kernel_spmd(nc, in_maps, core_ids=list(range(NCORES)), **spmd_kwargs)
    out = np.stack([res.results[b]["out"] for b in range(NCORES)], axis=0)
    return out.astype(np.float32), res


def kernel(x, wq, wk, wv, wo, bo):
    out, _ = run(x, wq, wk, wv, wo, bo)
    return out
